# revision 31
# baseline (speedup 1.0000x reference)
"""Batched EKF negative-log-likelihood loss on 8 Trainium2 NeuronCores.

Data-parallel Bass/Tile kernel: the N=8192 segments are sharded 1024 per
core, laid out as 128 partitions x 8 segments.  The EKF state is kept in
permuted + DT-scaled coordinates [x, y, th, DT*vx, DT*vy, DT*om] so the
transition Jacobian becomes F = [[I, I], [0, D]] with D = diag(d0, d1, a55)
- making F P F^T four strided elementwise ops.  The 3x3 innovation
covariance is inverted via a replicated 5x5 layout that yields the full
cofactor matrix in three vector ops.  tanh is computed via exp so that exp
and ln share one ScalarE activation-table set (no per-step table swaps).
"""

import os
import numpy as np

DT = 1.0 / 120.0
G = 9.81
K_SIGN = 100.0
TWO_PI = 2.0 * np.pi

N_CORES = 8
N_SEG = 8192
T_STEPS = 64
SEG_PER_CORE = N_SEG // N_CORES      # 1024
SLOTS = SEG_PER_CORE // 128          # 8 segments per partition

# ---------------------------------------------------------------------------
# Bass kernel builder
# ---------------------------------------------------------------------------


def _apx(tile_handle, off, *dims):
    """AP into `tile_handle` at element offset `off` with explicit
    (step, count) free dims, all 128 partitions."""
    from concourse.ap import AP

    base = tile_handle[:]
    ap = [list(base.ap[0])] + [[s, c] for (s, c) in dims]
    return AP(tensor=base.tensor, offset=base.offset + off, ap=ap)


def _build_ekf(tc, out_ap, meas_ap, x0_ap, cst_ap, sc, T, reps=1):
    import concourse.bacc as bacc
    import concourse.mybir as mybir

    _patch_act_tables(bacc, mybir)
    nc = tc.nc
    f32 = mybir.dt.float32
    A = mybir.AluOpType
    AF = mybir.ActivationFunctionType
    S = SLOTS  # 8

    with tc.tile_pool(name="persist", bufs=1) as pp, \
         tc.tile_pool(name="temps", bufs=2) as tp:
        MEAS = pp.tile([128, S * 3 * T], f32, tag="MEAS")
        X = pp.tile([128, S * 6], f32, tag="X")
        P = pp.tile([128, S * 36], f32, tag="P")
        T3 = pp.tile([128, S * 3], f32, tag="T3")
        CST = pp.tile([128, S * 9], f32, tag="CST")
        MPACC = pp.tile([128, 9 * S * T], f32, tag="MPACC")
        LDACC = pp.tile([128, T], f32, tag="LDACC")

        if meas_ap.shape[1] == S * 3 * T:
            nc.sync.dma_start(MEAS[:], meas_ap)
        else:  # timing variants with T < T_STEPS: strided slice of dram
            from concourse.ap import AP as _AP
            src = _AP(tensor=meas_ap.tensor, offset=meas_ap.offset,
                      ap=[list(meas_ap.ap[0]),
                          [meas_ap.shape[1] // S, S], [1, 3 * T]])
            nc.sync.dma_start(_apx(MEAS, 0, (3 * T, S), (1, 3 * T)), src)
        nc.sync.dma_start(CST[:], cst_ap)
        # T3 slot 2 stays 1.0 forever (makes d slot2 come out as a55)
        nc.vector.memset(_apx(T3, 2, (3, S), (1, 1)), 1.0)

        # reps>1 re-runs the whole filter (timing variants only)
        for _rep in range(reps):
            nc.sync.dma_start(X[:], x0_ap)
            # P0 = diag(.01, .01, .01, .01*DT^2 x3) per segment
            nc.vector.memset(P[:], 0.0)
            nc.vector.memset(_apx(P, 0, (36, S), (7, 3)), 0.01)
            nc.vector.memset(_apx(P, 21, (36, S), (7, 3)), sc["p0v"])
            _ekf_steps(nc, tc, tp, sc, T, S, mybir, A, AF,
                       MEAS, X, P, T3, CST, MPACC, LDACC)

        # ---- final: per-partition sum of maha + logdet ----
        MT = tp.tile([128, 1], f32, tag="MT")
        LT = tp.tile([128, 1], f32, tag="LT")
        TOT = tp.tile([128, 1], f32, tag="TOT")
        nc.vector.tensor_reduce(MT[:], MPACC[:], mybir.AxisListType.X, A.add)
        nc.vector.tensor_reduce(LT[:], LDACC[:], mybir.AxisListType.X, A.add)
        nc.vector.tensor_tensor(TOT[:], MT[:], LT[:], A.add)
        nc.sync.dma_start(out_ap, TOT[:])


def _ekf_steps(nc, tc, tp, sc, T, S, mybir, A, AF,
               MEAS, X, P, T3, CST, MPACC, LDACC):
    f32 = mybir.dt.float32
    if True:  # preserve indentation of the step loop
        for t in range(T):
            E = tp.tile([128, S * 2], f32, tag="E")
            EP = tp.tile([128, S * 2], f32, tag="EP")
            R2 = tp.tile([128, S * 2], f32, tag="R2")
            TSQ = tp.tile([128, S * 3], f32, tag="TSQ")
            D3 = tp.tile([128, S * 3], f32, tag="D3")
            Y = tp.tile([128, S * 3], f32, tag="Y")
            SEXT = tp.tile([128, S * 25], f32, tag="SEXT")
            PR1 = tp.tile([128, S * 9], f32, tag="PR1")
            PR2 = tp.tile([128, S * 9], f32, tag="PR2")
            C = tp.tile([128, S * 9], f32, tag="C")
            DP = tp.tile([128, S * 3], f32, tag="DP")
            DET = tp.tile([128, S], f32, tag="DET")
            REC = tp.tile([128, S], f32, tag="REC")
            LD = tp.tile([128, S], f32, tag="LD")
            SINV = tp.tile([128, S * 9], f32, tag="SINV")
            YY = tp.tile([128, S * 9], f32, tag="YY")
            KPR = tp.tile([128, S * 54], f32, tag="KPR")
            K6 = tp.tile([128, S * 18], f32, tag="K6")
            XP = tp.tile([128, S * 18], f32, tag="XP")
            XU = tp.tile([128, S * 6], f32, tag="XU")
            TMP0 = tp.tile([128, S * 36], f32, tag="TMP0")
            TMP1 = tp.tile([128, S * 36], f32, tag="TMP1")
            TMP2 = tp.tile([128, S * 36], f32, tag="TMP2")

            # ---- t = tanh(K*vel) via exp (stays in ln/exp table set) ----
            nc.scalar.activation(
                _apx(E, 0, (2, S), (1, 2)), _apx(X, 3, (6, S), (1, 2)),
                AF.Exp, scale=sc["kdt2"])
            nc.scalar.activation(
                _apx(EP, 0, (2, S), (1, 2)), _apx(E, 0, (2, S), (1, 2)),
                AF.Copy, bias=1.0)
            nc.vector.reciprocal(
                _apx(R2, 0, (2, S), (1, 2)), _apx(EP, 0, (2, S), (1, 2)))
            nc.scalar.activation(
                _apx(T3, 0, (3, S), (1, 2)), _apx(R2, 0, (2, S), (1, 2)),
                AF.Copy, bias=1.0, scale=-2.0)
            # d = cK2*t^2 + (a55 - cK2); slot2: t=1 -> d=a55
            nc.scalar.activation(
                _apx(TSQ, 0, (3, S), (1, 3)), _apx(T3, 0, (3, S), (1, 3)),
                AF.Square)
            nc.vector.tensor_scalar(
                _apx(D3, 0, (3, S), (1, 3)), _apx(TSQ, 0, (3, S), (1, 3)),
                sc["cK2"], sc["dcon"], A.mult, A.add)

            # ---- x_pred (on GPSIMD: off the DVE critical engine) ----
            # pos/th += vel'/om'   (uses OLD velocities)
            nc.gpsimd.tensor_tensor(
                _apx(X, 0, (6, S), (1, 3)), _apx(X, 0, (6, S), (1, 3)),
                _apx(X, 3, (6, S), (1, 3)), A.add)
            # vel' *= a55 (also om')
            nc.vector.tensor_scalar_mul(
                _apx(X, 3, (6, S), (1, 3)), _apx(X, 3, (6, S), (1, 3)),
                sc["a55"])
            # vel' -= cf2 * t   (slots 0,1 only)
            nc.vector.scalar_tensor_tensor(
                _apx(X, 3, (6, S), (1, 2)), _apx(T3, 0, (3, S), (1, 2)),
                -sc["cf2"], _apx(X, 3, (6, S), (1, 2)), A.mult, A.add)

            # ---- P_pred = F P F^T + Q ----
            nc.vector.tensor_tensor(  # top rows += bottom rows
                _apx(P, 0, (36, S), (1, 18)), _apx(P, 0, (36, S), (1, 18)),
                _apx(P, 18, (36, S), (1, 18)), A.add)
            nc.vector.tensor_tensor(  # bottom rows *= d (broadcast over cols)
                _apx(P, 18, (36, S), (6, 3), (1, 6)),
                _apx(P, 18, (36, S), (6, 3), (1, 6)),
                _apx(D3, 0, (3, S), (1, 3), (0, 6)), A.mult)
            nc.vector.tensor_tensor(  # left cols += right cols
                _apx(P, 0, (36, S), (6, 6), (1, 3)),
                _apx(P, 0, (36, S), (6, 6), (1, 3)),
                _apx(P, 3, (36, S), (6, 6), (1, 3)), A.add)
            nc.vector.tensor_tensor(  # right cols *= d (broadcast over rows)
                _apx(P, 3, (36, S), (6, 6), (1, 3)),
                _apx(P, 3, (36, S), (6, 6), (1, 3)),
                _apx(D3, 0, (3, S), (0, 6), (1, 3)), A.mult)
            nc.vector.tensor_tensor(  # diag += q
                _apx(P, 0, (36, S), (7, 6)), _apx(P, 0, (36, S), (7, 6)),
                _apx(CST, 3, (9, S), (1, 6)), A.add)

            # ---- innovation y = z - Hx, wrap theta ----
            nc.vector.tensor_tensor(
                _apx(Y, 0, (3, S), (1, 3)), _apx(MEAS, 3 * t, (3 * T, S), (1, 3)),
                _apx(X, 0, (6, S), (1, 3)), A.subtract)
            nc.vector.add_range_wrap(
                _apx(Y, 2, (3, S), (1, 1)), _apx(Y, 2, (3, S), (1, 1)),
                0.0, 1.5 * np.pi, TWO_PI)

            # ---- S in replicated 5x5 layout: sext[u,v] = S[u%3, v%3] ----
            nc.vector.tensor_tensor(  # diag = P diag + r
                _apx(SEXT, 0, (25, S), (6, 3)), _apx(P, 0, (36, S), (7, 3)),
                _apx(CST, 0, (9, S), (1, 3)), A.add)
            nc.scalar.copy(  # (0,1),(0,2) = P1,P2
                _apx(SEXT, 1, (25, S), (1, 2)), _apx(P, 1, (36, S), (1, 2)))
            nc.scalar.copy(  # (1,0),(2,0) = P1,P2
                _apx(SEXT, 5, (25, S), (5, 2)), _apx(P, 1, (36, S), (1, 2)))
            nc.scalar.copy(  # (1,2),(2,1) = P8
                _apx(SEXT, 7, (25, S), (4, 2)), _apx(P, 8, (36, S), (0, 2)))
            nc.scalar.copy(  # cols 3,4 = cols 0,1 (rows 0..2)
                _apx(SEXT, 3, (25, S), (5, 3), (1, 2)),
                _apx(SEXT, 0, (25, S), (5, 3), (1, 2)))
            nc.scalar.copy(  # rows 3,4 = rows 0,1
                _apx(SEXT, 15, (25, S), (1, 10)), _apx(SEXT, 0, (25, S), (1, 10)))

            # ---- cofactor matrix C[a,b] = s[a+1,b+1]s[a+2,b+2]-s[a+1,b+2]s[a+2,b+1]
            nc.vector.tensor_tensor(
                _apx(PR1, 0, (9, S), (1, 9)), _apx(SEXT, 6, (25, S), (5, 3), (1, 3)),
                _apx(SEXT, 12, (25, S), (5, 3), (1, 3)), A.mult)
            nc.gpsimd.tensor_tensor(
                _apx(PR2, 0, (9, S), (1, 9)), _apx(SEXT, 7, (25, S), (5, 3), (1, 3)),
                _apx(SEXT, 11, (25, S), (5, 3), (1, 3)), A.mult)
            nc.vector.tensor_tensor(
                _apx(C, 0, (9, S), (1, 9)), _apx(PR1, 0, (9, S), (1, 9)),
                _apx(PR2, 0, (9, S), (1, 9)), A.subtract)

            # ---- det, 1/det, ln(det) ----
            nc.vector.tensor_tensor(
                _apx(DP, 0, (3, S), (1, 3)), _apx(SEXT, 0, (25, S), (1, 3)),
                _apx(C, 0, (9, S), (1, 3)), A.mult)
            nc.vector.tensor_reduce(
                _apx(DET, 0, (1, S)), _apx(DP, 0, (3, S), (1, 3)),
                mybir.AxisListType.X, A.add)
            nc.vector.reciprocal(_apx(REC, 0, (1, S)), _apx(DET, 0, (1, S)))
            nc.scalar.activation(
                _apx(LD, 0, (1, S)), _apx(DET, 0, (1, S)), AF.Ln,
                accum_out=LDACC[:, t:t + 1])

            # ---- Sinv = C / det ----
            nc.vector.tensor_tensor(
                _apx(SINV, 0, (9, S), (1, 9)), _apx(C, 0, (9, S), (1, 9)),
                _apx(REC, 0, (1, S), (0, 9)), A.mult)

            # ---- maha contributions yT Sinv y into MPACC (9 per seg) ----
            nc.vector.tensor_tensor(
                _apx(YY, 0, (9, S), (3, 3), (1, 3)),
                _apx(Y, 0, (3, S), (1, 3), (0, 3)),
                _apx(Y, 0, (3, S), (0, 3), (1, 3)), A.mult)
            nc.vector.tensor_tensor(
                _apx(MPACC, 9 * S * t, (9, S), (1, 9)),
                _apx(SINV, 0, (9, S), (1, 9)),
                _apx(YY, 0, (9, S), (1, 9)), A.mult)

            # ---- K = Pc Sinv, stored transposed: K6[a,i] = K[i,a] ----
            for a in range(3):
                nc.vector.tensor_tensor(
                    _apx(KPR, 18 * a, (54, S), (3, 6), (1, 3)),
                    _apx(P, 0, (36, S), (6, 6), (1, 3)),
                    _apx(SINV, a, (9, S), (0, 6), (3, 3)), A.mult)
            for a in range(3):
                nc.vector.tensor_reduce(
                    _apx(K6, 6 * a, (18, S), (1, 6)),
                    _apx(KPR, 18 * a, (54, S), (3, 6), (1, 3)),
                    mybir.AxisListType.X, A.add)

            # ---- x_new = x_pred + K y ----
            nc.vector.tensor_tensor(
                _apx(XP, 0, (18, S), (3, 6), (1, 3)),
                _apx(K6, 0, (18, S), (1, 6), (6, 3)),
                _apx(Y, 0, (3, S), (0, 6), (1, 3)), A.mult)
            nc.vector.tensor_reduce(
                _apx(XU, 0, (6, S), (1, 6)), _apx(XP, 0, (18, S), (3, 6), (1, 3)),
                mybir.AxisListType.X, A.add)
            nc.gpsimd.tensor_tensor(
                _apx(X, 0, (6, S), (1, 6)), _apx(X, 0, (6, S), (1, 6)),
                _apx(XU, 0, (6, S), (1, 6)), A.add)

            # ---- P_new = P_pred - sum_a K[:,a] (x) Pc[:,a]^T ----
            # tmp_a[i,j] = K6[a,i] * P[j,a]; a=1,2 products + their sum on
            # GPSIMD, leaving DVE with one product and two subtracts.
            nc.vector.tensor_tensor(
                _apx(TMP0, 0, (36, S), (6, 6), (1, 6)),
                _apx(K6, 0, (18, S), (1, 6), (0, 6)),
                _apx(P, 0, (36, S), (0, 6), (6, 6)), A.mult)
            nc.gpsimd.tensor_tensor(
                _apx(TMP1, 0, (36, S), (6, 6), (1, 6)),
                _apx(K6, 6, (18, S), (1, 6), (0, 6)),
                _apx(P, 1, (36, S), (0, 6), (6, 6)), A.mult)
            nc.gpsimd.tensor_tensor(
                _apx(TMP2, 0, (36, S), (6, 6), (1, 6)),
                _apx(K6, 12, (18, S), (1, 6), (0, 6)),
                _apx(P, 2, (36, S), (0, 6), (6, 6)), A.mult)
            nc.gpsimd.tensor_tensor(  # S12 = TMP1 + TMP2 (into TMP1)
                _apx(TMP1, 0, (36, S), (1, 36)), _apx(TMP1, 0, (36, S), (1, 36)),
                _apx(TMP2, 0, (36, S), (1, 36)), A.add)
            nc.vector.tensor_tensor(
                _apx(P, 0, (36, S), (1, 36)), _apx(P, 0, (36, S), (1, 36)),
                _apx(TMP0, 0, (36, S), (1, 36)), A.subtract)
            nc.vector.tensor_tensor(
                _apx(P, 0, (36, S), (1, 36)), _apx(P, 0, (36, S), (1, 36)),
                _apx(TMP1, 0, (36, S), (1, 36)), A.subtract)


def _derived_scalars(params, covariance_params):
    dyna = np.abs(params.astype(np.float64))
    fric, damp = float(dyna[0]), float(dyna[1])
    cp = covariance_params.astype(np.float64)
    a55 = 1.0 - DT * damp
    cK2 = DT * fric * G * K_SIGN
    sc = {
        "kdt2": 2.0 * K_SIGN / DT,
        "a55": a55,
        "cf2": DT * DT * fric * G,
        "cK2": cK2,
        "dcon": a55 - cK2,
        "p0v": 0.01 * DT * DT,
    }
    r_s = np.exp(cp[0:3])
    q_s = np.array([
        np.exp(cp[3]), np.exp(cp[3]), np.exp(cp[5]),
        np.exp(cp[4]) * DT * DT, np.exp(cp[4]) * DT * DT,
        np.exp(cp[6]) * DT * DT,
    ])
    return sc, r_s.astype(np.float32), q_s.astype(np.float32)


# ---------------------------------------------------------------------------
# Host-side runner (compiled callable cached across kernel() calls)
# ---------------------------------------------------------------------------

_CACHE = {}
LAST_RESULT = None


_ACT_PATCHED = False


def _patch_act_tables(bacc, mybir):
    """Make the act-table-load pass keep Exp and Ln only in the combined
    natural_log_exp_and_others set.  The default chooser picks the first
    set containing each function, which ping-pongs exp_and_others <->
    natural_log on every EKF step (~5.3us/step of table loads).  Set
    positions are preserved so act_func_set_id stays valid."""
    global _ACT_PATCHED
    if _ACT_PATCHED:
        return
    orig = bacc.get_activation_tables
    AF = mybir.ActivationFunctionType

    def patched(module_arch):
        tables = orig(module_arch)
        for name, funcs in tables.items():
            if name != "natural_log_exp_and_others":
                funcs.discard(AF.Exp)
                funcs.discard(AF.Ln)
        return tables

    bacc.get_activation_tables = patched
    _ACT_PATCHED = True


def _get_runner(key, sc, r_s, q_s, T=T_STEPS, reps=1):
    """Build the Bass kernel and a persistent jitted SPMD callable.

    run_bass_kernel_spmd re-creates its jax.jit wrapper on every call, which
    re-traces and hits compile caches each time; for a fast warm path we
    build the sharded executable once and reuse it.
    """
    key = (key, T, reps)
    if key in _CACHE:
        return _CACHE[key]

    import jax
    from jax.sharding import Mesh, PartitionSpec
    from jax.experimental.shard_map import shard_map
    import concourse.bacc as bacc
    import concourse.mybir as mybir
    import concourse.tile as tile
    from concourse import bass2jax

    f32 = mybir.dt.float32
    nc = bacc.Bacc("TRN2", target_bir_lowering=False, debug=False)
    meas_d = nc.dram_tensor("meas", [128, SLOTS * 3 * T_STEPS], f32,
                            kind="ExternalInput")
    x0_d = nc.dram_tensor("x0", [128, SLOTS * 6], f32, kind="ExternalInput")
    cst_d = nc.dram_tensor("cst", [128, SLOTS * 9], f32, kind="ExternalInput")
    out_d = nc.dram_tensor("out", [128, 1], f32, kind="ExternalOutput")
    with tile.TileContext(nc) as tc:
        _build_ekf(tc, out_d.ap(), meas_d.ap(), x0_d.ap(), cst_d.ap(), sc, T,
                   reps=reps)
    nc.compile()

    bass2jax.install_neuronx_cc_hook()
    out_aval = jax.core.ShapedArray((128, 1), np.float32)
    pid_name = (nc.partition_id_tensor.name if nc.partition_id_tensor
                else None)
    in_names = ("meas", "x0", "cst", "out") + (
        (pid_name,) if pid_name else ())

    def _body(meas, x0, cst, out_zero):
        operands = [meas, x0, cst, out_zero]
        if pid_name:
            operands.append(bass2jax.partition_id_tensor())
        outs = bass2jax._bass_exec_p.bind(
            *operands,
            out_avals=(out_aval,),
            in_names=in_names,
            out_names=("out",),
            lowering_input_output_aliases=(),
            sim_require_finite=True,
            sim_require_nnan=True,
            nc=nc,
        )
        return outs[0]

    devices = jax.devices()[:N_CORES]
    mesh = Mesh(np.asarray(devices), ("core",))
    sharded = jax.jit(
        shard_map(_body, mesh=mesh,
                  in_specs=(PartitionSpec("core"),) * 4,
                  out_specs=PartitionSpec("core"), check_rep=False),
        donate_argnums=(3,),
        keep_unused=True,
    )
    _CACHE[key] = (nc, sharded)
    return _CACHE[key]


def make_chain_runner(nc, k):
    """Jitted callable that executes the NEFF k times back-to-back inside
    one dispatch (serialized via the out-buffer dependency).  Used to
    measure pure device execution time without per-call RPC overhead."""
    import jax
    from jax.sharding import Mesh, PartitionSpec
    from jax.experimental.shard_map import shard_map
    from concourse import bass2jax

    out_aval = jax.core.ShapedArray((128, 1), np.float32)
    pid_name = (nc.partition_id_tensor.name if nc.partition_id_tensor
                else None)
    in_names = ("meas", "x0", "cst", "out") + (
        (pid_name,) if pid_name else ())

    def _body(meas, x0, cst, out_zero):
        out = out_zero
        for _ in range(k):
            operands = [meas, x0, cst, out]
            if pid_name:
                operands.append(bass2jax.partition_id_tensor())
            out = bass2jax._bass_exec_p.bind(
                *operands,
                out_avals=(out_aval,),
                in_names=in_names,
                out_names=("out",),
                lowering_input_output_aliases=(),
                sim_require_finite=True,
                sim_require_nnan=True,
                nc=nc,
            )[0]
        return out

    devices = jax.devices()[:N_CORES]
    mesh = Mesh(np.asarray(devices), ("core",))
    return jax.jit(
        shard_map(_body, mesh=mesh,
                  in_specs=(PartitionSpec("core"),) * 4,
                  out_specs=PartitionSpec("core"), check_rep=False),
        donate_argnums=(3,),
        keep_unused=True,
    )


def _prep_inputs_global(init_state, measurements, r_s, q_s):
    """Concatenated-over-cores inputs for the shard_map callable."""
    cst_row = np.concatenate([r_s, q_s]).astype(np.float32)          # 9
    cst = np.tile(cst_row, (N_CORES * 128, SLOTS))                   # [1024, 72]
    x_perm = init_state[:, [0, 1, 4, 2, 3, 5]].copy()
    x_perm[:, 3:6] *= DT
    meas_g = np.ascontiguousarray(measurements).reshape(
        N_CORES * 128, SLOTS * 3 * T_STEPS)
    x0_g = np.ascontiguousarray(x_perm).reshape(N_CORES * 128, SLOTS * 6)
    return meas_g, x0_g, cst


def kernel(params, covariance_params, init_state, measurements):
    global LAST_RESULT
    params = np.asarray(params, dtype=np.float32)
    covariance_params = np.asarray(covariance_params, dtype=np.float32)
    init_state = np.asarray(init_state, dtype=np.float32)
    measurements = np.asarray(measurements, dtype=np.float32)

    if (init_state.shape != (N_SEG, 6)
            or measurements.shape != (N_SEG, T_STEPS, 3)
            or params.shape != (4,) or covariance_params.shape != (7,)):
        return np.float32(_ekf_numpy(params, covariance_params, init_state,
                                     measurements) / init_state.shape[0])

    try:
        sc, r_s, q_s = _derived_scalars(params, covariance_params)
        key = (params.tobytes(), covariance_params.tobytes())
        nc, sharded = _get_runner(key, sc, r_s, q_s)
        meas_g, x0_g, cst_g = _prep_inputs_global(init_state, measurements,
                                                 r_s, q_s)
        out_zero = np.zeros((N_CORES * 128, 1), np.float32)
        out = np.asarray(sharded(meas_g, x0_g, cst_g, out_zero))
        LAST_RESULT = out
        total = np.sum(out.astype(np.float64))
        return np.float32(0.5 * total / N_SEG)
    except Exception:
        if os.environ.get("EKF_NO_FALLBACK"):
            raise
        return np.float32(_ekf_numpy(params, covariance_params, init_state,
                                     measurements) / init_state.shape[0])


# ---------------------------------------------------------------------------
# Pure-numpy fallback (reference-equivalent)
# ---------------------------------------------------------------------------


def _ekf_numpy(params, covariance_params, init_state, measurements):
    dyna = np.abs(params).astype(np.float32)
    fric, damp = dyna[0], dyna[1]
    cp = covariance_params
    R = np.diag(np.exp(cp[:3])).astype(np.float32)
    Q = np.diag(np.exp(np.stack(
        [cp[3], cp[3], cp[4], cp[4], cp[5], cp[6]]))).astype(np.float32)
    N = init_state.shape[0]
    midx = [0, 1, 4]
    x = init_state.copy()
    P = np.broadcast_to(np.eye(6, dtype=np.float32) * 0.01, (N, 6, 6)).copy()
    I6 = np.eye(6, dtype=np.float32)
    total = np.float64(0.0)
    for ti in range(measurements.shape[1]):
        z = measurements[:, ti, :]
        vel = x[:, 2:4]
        t = np.tanh(K_SIGN * vel)
        x_pred = np.concatenate(
            [x[:, 0:2] + DT * vel,
             vel - DT * (damp * vel + fric * G * t),
             x[:, 4:5] + DT * x[:, 5:6],
             x[:, 5:6] - DT * damp * x[:, 5:6]], axis=1).astype(np.float32)
        dv = 1.0 - DT * (damp + fric * G * K_SIGN * (1.0 - t * t))
        F = np.broadcast_to(I6, (N, 6, 6)).copy()
        F[:, 0, 2] = DT; F[:, 1, 3] = DT; F[:, 4, 5] = DT
        F[:, 2, 2] = dv[:, 0]; F[:, 3, 3] = dv[:, 1]
        F[:, 5, 5] = 1.0 - DT * damp
        P = np.einsum('nij,njk,nlk->nil', F, P, F) + Q
        y = z - x_pred[:, midx]
        ang = y[:, 2]
        ang = np.where(ang > 1.5 * np.pi, ang - TWO_PI,
                       np.where(ang < -1.5 * np.pi, ang + TWO_PI, ang))
        y[:, 2] = ang
        Sm = P[:, midx][:, :, midx] + R
        Sinv = np.linalg.inv(Sm.astype(np.float64)).astype(np.float32)
        K = np.einsum('nij,njk->nik', P[:, :, midx], Sinv)
        x = x_pred + np.einsum('nij,nj->ni', K, y)
        KH = np.zeros((N, 6, 6), np.float32)
        KH[:, :, midx] = K
        P = np.einsum('nij,njk->nik', I6 - KH, P)
        sign, logdet = np.linalg.slogdet(Sm.astype(np.float64))
        maha = np.einsum('ni,nij,nj->n', y, Sinv, y)
        total += 0.5 * np.sum(logdet + maha)
    return total
